# revision 27
# baseline (speedup 1.0000x reference)
"""Disentangled spatial attention TRN2 kernel (8 NeuronCores).

Sharding: 8 cores = 2 batches x 4 head-groups (4 heads each).
Per core, transposed-activation layout:
  qcat[h] (128, L):  rows 0:64 qt_h, rows 64:128 qs_h
  kcat[h] (128, L):  rows 0:64 k1_h = kt + lam_ts*ks,
                     rows 64:128 k2_h = lam_st*kt + lam_ss*ks
  scores^T chunk = kcat_chunk.T @ qcat  (both reference score einsums
  fused into one K=128 matmul; lam_* folded into weight shards on host)
  softmax row-sums ride along the PV matmul as 64 replicated "ones"
  columns of the v operand; normalization happens on the way into the
  transposed y layout that feeds the output projection.
Phase-1 matmuls run in float32r; attention + projection operands are
bf16 (fp32 PSUM accumulation).  v/c biases are folded in on the host
(exact: softmax rows sum to 1), qkv biases are added on device.
"""
import numpy as np
import ml_dtypes
import concourse.bass as bass
import concourse.mybir as mybir
import concourse.tile as tile
from concourse.bass_utils import run_bass_kernel_spmd

F32 = mybir.dt.float32
F32R = mybir.dt.float32r
BF16 = mybir.dt.float16  # fp16: same PE rate as bf16, 8x lower rounding error
AF = mybir.ActivationFunctionType

B, L, E, H, D = 2, 2048, 1024, 16, 64
HPC = 4          # heads per core
NCORES = 8
LTB = 512        # L block for phase 1
NLTB = L // LTB  # 4
NCHUNK = L // 128  # 16 Lk chunks
EC = E // 128    # 8 E chunks


def _split_multi_waits(nc, max_waits=1):
    """walrus codegen allows only one sync wait per instruction; move extra
    waits onto standalone same-engine NoOps placed just before."""
    n_split = 0
    for f in nc.m.functions:
        for blk in f.blocks:
            insts = list(blk.instructions)
            out = []
            changed = False
            for inst in insts:
                si = inst.sync_info
                waits = list(si.on_wait) if si is not None and si.on_wait else []
                if len(waits) > max_waits:
                    keep = waits[-max_waits:]
                    extra = waits[:-max_waits]
                    for w in extra:
                        nop = mybir.InstNoOp(
                            name=f"{inst.name}-wsplit{n_split}",
                            engine=inst.engine,
                            ins=[], outs=[],
                            sync_info=mybir.SyncInfo(on_wait=[w], on_update=[]),
                        )
                        out.append(nop)
                        n_split += 1
                    inst.sync_info = mybir.SyncInfo(
                        on_wait=keep,
                        on_update=list(si.on_update) if si.on_update else [],
                    )
                    changed = True
                out.append(inst)
            if changed:
                blk.instructions = out
    return n_split


def _build():
    nc = bass.Bass()
    xtT = nc.declare_dram_parameter("xtT", [E, L], BF16, isOutput=False)
    xsT = nc.declare_dram_parameter("xsT", [E, L], BF16, isOutput=False)
    wq = nc.declare_dram_parameter("wq", [128, EC, HPC * D], BF16, isOutput=False)
    wqs = nc.declare_dram_parameter("wqs", [128, EC, HPC * D], BF16, isOutput=False)
    wkt = nc.declare_dram_parameter("wkt", [128, EC, HPC * D], BF16, isOutput=False)
    wks = nc.declare_dram_parameter("wks", [128, EC, HPC * D], BF16, isOutput=False)
    wv = nc.declare_dram_parameter("wv", [128, EC, HPC * D], BF16, isOutput=False)
    wc = nc.declare_dram_parameter("wc", [128, 2, E], BF16, isOutput=False)
    bq = nc.declare_dram_parameter("bq", [128, 2], F32, isOutput=False)
    bqs = nc.declare_dram_parameter("bqs", [128, 2], F32, isOutput=False)
    bk1 = nc.declare_dram_parameter("bk1", [128, 2], F32, isOutput=False)
    bk2 = nc.declare_dram_parameter("bk2", [128, 2], F32, isOutput=False)
    lamv = nc.declare_dram_parameter("lamv", [128, 3], F32, isOutput=False)
    ones = nc.declare_dram_parameter("ones", [128, NCHUNK, 2, 64], BF16,
                                     isOutput=False)
    out = nc.declare_dram_parameter("out", [L, E], F32, isOutput=True)

    xtT_v = xtT.rearrange("(k p) l -> p k l", p=128)   # (128, 8, L)
    xsT_v = xsT.rearrange("(k p) l -> p k l", p=128)

    with tile.TileContext(nc) as tc:
        with tc.tile_pool(name="wpool", bufs=1) as wpool, \
             tc.tile_pool(name="persist", bufs=1) as pp:
            qcat = [pp.tile([128, L], BF16, tag=f"qcat{h}", name=f"qcat{h}")
                    for h in range(HPC)]
            kcat = [pp.tile([128, L], BF16, tag=f"kcat{h}", name=f"kcat{h}")
                    for h in range(HPC)]
            # v_aug: (128, chunk, head, 128); head slot s=0: [ones | v],
            # s=1: [v | ones]
            v_sb = pp.tile([128, NCHUNK, HPC, 128], BF16, name="v_sb")
            yT = [pp.tile([128, L], BF16, tag=f"yT{j}", name=f"yT{j}")
                  for j in range(2)]
            xt_sb = pp.tile([128, EC, L], BF16, name="xt_sb")
            xs_sb = pp.tile([128, EC, L], BF16, name="xs_sb")

            wq_sb = wpool.tile([128, EC, HPC * D], BF16)
            wqs_sb = wpool.tile([128, EC, HPC * D], BF16)
            wkt_sb = wpool.tile([128, EC, HPC * D], BF16)
            wks_sb = wpool.tile([128, EC, HPC * D], BF16)
            wv_sb = wpool.tile([128, EC, HPC * D], BF16)
            bq_sb = wpool.tile([128, 2], F32)
            bqs_sb = wpool.tile([128, 2], F32)
            bk1_sb = wpool.tile([128, 2], F32)
            bk2_sb = wpool.tile([128, 2], F32)
            lam_sb = wpool.tile([128, 3], F32)
            wc_sb = wpool.tile([128, 2, E], BF16)

            nc.sync.dma_start(wv_sb[:], wv[:])
            for xc in range(4):
                xls = slice(xc * 512, (xc + 1) * 512)
                nc.sync.dma_start(xt_sb[:, :, xls], xtT_v[:, :, xls])
            nc.sync.dma_start(wkt_sb[:], wkt[:])
            nc.sync.dma_start(wks_sb[:], wks[:])
            nc.sync.dma_start(lam_sb[:], lamv[:])
            nc.sync.dma_start(bk1_sb[:], bk1[:])
            nc.sync.dma_start(bk2_sb[:], bk2[:])
            for xc in range(4):
                xls = slice(xc * 512, (xc + 1) * 512)
                nc.sync.dma_start(xs_sb[:, :, xls], xsT_v[:, :, xls])
            nc.sync.dma_start(wq_sb[:], wq[:])
            nc.sync.dma_start(bq_sb[:], bq[:])
            nc.sync.dma_start(wqs_sb[:], wqs[:])
            nc.sync.dma_start(bqs_sb[:], bqs[:])
            nc.sync.dma_start(v_sb[:, :, 0::2, 0:64], ones[:])
            nc.sync.dma_start(v_sb[:, :, 1::2, 64:128], ones[:])
            nc.sync.dma_start(wc_sb[:], wc[:])

            # ---- head pairs: QKV then attention, interleaved ----
            with tc.tile_pool(name="expp", bufs=6) as expp, \
                 tc.tile_pool(name="np2", bufs=2) as np2, \
                 tc.tile_pool(name="kcp", bufs=3) as kcp, \
                 tc.tile_pool(name="p2s", bufs=2, space="PSUM") as p2s, \
                 tc.tile_pool(name="p2y", bufs=2, space="PSUM") as p2y:
                pvp_cm = tc.tile_pool(name="pvp", bufs=3, space="PSUM")
                pvp = pvp_cm.__enter__()
                M_ = mybir.AluOpType.mult
                A_ = mybir.AluOpType.add

                def emit_ktks(j):
                    # kt/ks for the pair; combine into kcat (k1 | k2) with
                    # lam scalars; per-lt staged DMAs for partition shifts
                    for lt in range(4):
                        ls = slice(lt * 512, (lt + 1) * 512)
                        ktp = pvp.tile([128, 512], F32, tag="p1",
                                       name=f"ktp{j}{lt}")
                        for k in range(EC):
                            nc.tensor.matmul(
                                ktp[:], wkt_sb[:, k, j * 128:(j + 1) * 128],
                                xt_sb[:, k, ls],
                                start=(k == 0), stop=(k == EC - 1),
                                skip_group_check=True)
                        ksp = pvp.tile([128, 512], F32, tag="p1",
                                       name=f"ksp{j}{lt}")
                        for k in range(EC):
                            nc.tensor.matmul(
                                ksp[:], wks_sb[:, k, j * 128:(j + 1) * 128],
                                xs_sb[:, k, ls],
                                start=(k == 0), stop=(k == EC - 1),
                                skip_group_check=True)
                        kt1 = kcp.tile([128, 512], F32, tag="kt1",
                                       name=f"kt1{j}{lt}")
                        nc.scalar.activation(kt1[:], ktp[:], AF.Identity,
                                             bias=bk1_sb[:, j:j + 1])
                        kt2 = kcp.tile([128, 512], F32, tag="kt2",
                                       name=f"kt2{j}{lt}")
                        nc.scalar.activation(
                            kt2[:], ktp[:], AF.Identity,
                            bias=bk2_sb[:, j:j + 1], scale=lam_sb[:, 1:2])
                        # k1 (both heads) and k2 (both heads), full width
                        k1s = kcp.tile([128, 512], BF16, tag="k1s",
                                       name=f"k1s{j}{lt}")
                        nc.vector.scalar_tensor_tensor(
                            k1s[:], ksp[:], lam_sb[:, 0:1], kt1[:], M_, A_)
                        k2s = kcp.tile([128, 512], BF16, tag="k2s",
                                       name=f"k2s{j}{lt}")
                        nc.vector.scalar_tensor_tensor(
                            k2s[:], ksp[:], lam_sb[:, 2:3], kt2[:], M_, A_)
                        nc.gpsimd.dma_start(kcat[2 * j][0:64, ls], k1s[0:64, :])
                        nc.gpsimd.dma_start(kcat[2 * j + 1][0:64, ls],
                                            k1s[64:128, :])
                        nc.gpsimd.dma_start(kcat[2 * j][64:128, ls], k2s[0:64, :])
                        nc.gpsimd.dma_start(kcat[2 * j + 1][64:128, ls],
                                            k2s[64:128, :])

                def emit_q(j):
                    for lt in range(4):
                        ls = slice(lt * 512, (lt + 1) * 512)
                        pq = pvp.tile([128, 512], F32, tag="p1",
                                      name=f"pq{j}{lt}")
                        for k in range(EC):
                            nc.tensor.matmul(
                                pq[:], wq_sb[:, k, j * 128:(j + 1) * 128],
                                xt_sb[:, k, ls],
                                start=(k == 0), stop=(k == EC - 1),
                                skip_group_check=True)
                        nc.vector.tensor_scalar_add(
                            qcat[2 * j][0:64, ls], pq[0:64, :],
                            bq_sb[0:64, j:j + 1])
                        qst = kcp.tile([128, 512], BF16, tag="qst",
                                       name=f"qst{j}{lt}")
                        nc.vector.tensor_scalar_add(
                            qst[64:128, :], pq[64:128, :],
                            bq_sb[64:128, j:j + 1])
                        nc.gpsimd.dma_start(qcat[2 * j + 1][0:64, ls],
                                            qst[64:128, :])
                    for lt in range(4):
                        ls = slice(lt * 512, (lt + 1) * 512)
                        pq = pvp.tile([128, 512], F32, tag="p1",
                                      name=f"pqs{j}{lt}")
                        for k in range(EC):
                            nc.tensor.matmul(
                                pq[:], wqs_sb[:, k, j * 128:(j + 1) * 128],
                                xs_sb[:, k, ls],
                                start=(k == 0), stop=(k == EC - 1),
                                skip_group_check=True)
                        qst = kcp.tile([128, 512], BF16, tag="qst",
                                       name=f"qsst{j}{lt}")
                        nc.scalar.activation(
                            qst[0:64, :], pq[0:64, :], AF.Identity,
                            bias=bqs_sb[0:64, j:j + 1])
                        nc.gpsimd.dma_start(qcat[2 * j][64:128, ls],
                                            qst[0:64, :])
                        nc.scalar.activation(
                            qcat[2 * j + 1][64:128, ls], pq[64:128, :],
                            AF.Identity, bias=bqs_sb[64:128, j:j + 1])

                def emit_v():
                    for ck in range(NCHUNK):
                        pv = pvp.tile([128, HPC * D], F32, tag="p1",
                                      name=f"pv{ck}")
                        for k in range(EC):
                            nc.tensor.matmul(
                                pv[:], xt_sb[:, k, ck * 128:(ck + 1) * 128],
                                wv_sb[:, k, :],
                                start=(k == 0), stop=(k == EC - 1),
                                skip_group_check=True)
                        pv_v = pv.rearrange("p (h d) -> p h d", d=D)
                        nc.vector.tensor_copy(v_sb[:, ck, 0::2, 64:128],
                                              pv_v[:, 0::2, :])
                        nc.vector.tensor_copy(v_sb[:, ck, 1::2, 0:64],
                                              pv_v[:, 1::2, :])

                def emit_attn(h, lqs_list=range(4)):
                    j, s = h // 2, h % 2
                    sums_h = slice(0, 64) if s == 0 else slice(64, 128)
                    y_h = slice(64, 128) if s == 0 else slice(0, 64)
                    slot = slice(0, 64) if s == 0 else slice(64, 128)
                    for lq in lqs_list:
                        qs_ = slice(lq * 512, (lq + 1) * 512)
                        py = p2y.tile([128, 512], F32, tag="py", bufs=1,
                                      name=f"py{h}{lq}")
                        for g in range(8):
                            ps = p2s.tile([128, 1024], F32, tag="ps",
                                          name=f"ps{h}{lq}{g}")
                            for hf in range(2):
                                ck = 2 * g + hf
                                nc.tensor.matmul(
                                    ps[:, hf * 512:(hf + 1) * 512],
                                    kcat[h][:, ck * 128:(ck + 1) * 128],
                                    qcat[h][:, qs_],
                                    start=True, stop=True,
                                    skip_group_check=True)
                            ex = expp.tile([128, 1024], BF16, tag="ex",
                                           name=f"ex{h}{lq}{g}")
                            nc.scalar.activation(ex[:], ps[:], AF.Exp,
                                                 scale=0.125)
                            for hf in range(2):
                                ck = 2 * g + hf
                                nc.tensor.matmul(
                                    py[:], v_sb[:, ck, h, :],
                                    ex[:, hf * 512:(hf + 1) * 512],
                                    start=(ck == 0), stop=(ck == NCHUNK - 1),
                                    skip_group_check=True)
                        ysb = np2.tile([128, 512], F32, tag="ysb",
                                       name=f"ysb{h}{lq}")
                        rec = np2.tile([128, 512], F32, tag="rec",
                                       name=f"rec{h}{lq}")
                        nc.vector.tensor_copy(ysb[:], py[:])
                        if h == 3:
                            lnt = np2.tile([128, 512], F32, tag="lnt",
                                           name=f"ln{h}{lq}")
                            nc.scalar.activation(lnt[sums_h, :],
                                                 ysb[sums_h, :], AF.Ln)
                            nc.scalar.activation(rec[sums_h, :],
                                                 lnt[sums_h, :], AF.Exp,
                                                 scale=-1.0)
                        else:
                            nc.vector.reciprocal(rec[sums_h, :],
                                                 ysb[sums_h, :])
                        rec2 = np2.tile([128, 512], F32, tag="rec2",
                                        name=f"rec2{h}{lq}")
                        nc.gpsimd.dma_start(rec2[y_h, :], rec[sums_h, :])
                        yst = np2.tile([128, 512], BF16, tag="yst",
                                       name=f"yst{h}{lq}")
                        nc.vector.tensor_tensor(yst[y_h, :], ysb[y_h, :],
                                                rec2[y_h, :],
                                                mybir.AluOpType.mult)
                        nc.gpsimd.dma_start(yT[j][slot, qs_], yst[y_h, :])

                emit_v()
                emit_ktks(0)
                emit_q(0)
                emit_attn(0)
                emit_attn(1)
                emit_ktks(1)
                emit_q(1)
                pvp_cm.__exit__(None, None, None)
                emit_attn(2)

                with tc.tile_pool(name="outp", bufs=3) as outp, \
                     tc.tile_pool(name="p3o", bufs=2, space="PSUM") as p3o:
                    def emit_proj(lq):
                        for lqt in range(lq * 4, (lq + 1) * 4):
                            lqs = slice(lqt * 128, (lqt + 1) * 128)
                            ot = outp.tile([128, E], F32, tag="ot",
                                           name=f"ot{lqt}")
                            for nch in range(2):
                                ns = slice(nch * 512, (nch + 1) * 512)
                                po = p3o.tile([128, 512], F32, tag="po",
                                              name=f"po{lqt}{nch}")
                                nc.tensor.matmul(po[:], yT[0][:, lqs],
                                                 wc_sb[:, 0, ns],
                                                 start=True, stop=False,
                                                 skip_group_check=True)
                                nc.tensor.matmul(po[:], yT[1][:, lqs],
                                                 wc_sb[:, 1, ns],
                                                 start=False, stop=True,
                                                 skip_group_check=True)
                                if nch == 0:
                                    nc.scalar.copy(ot[:, ns], po[:])
                                else:
                                    nc.vector.tensor_copy(ot[:, ns], po[:])
                            nc.sync.dma_start(out[lqs, :], ot[:])

                    for lq in range(4):
                        emit_attn(3, [lq])
                        emit_proj(lq)

    return nc


_NC_CACHE = None


def _get_nc():
    global _NC_CACHE
    if _NC_CACHE is None:
        nc = _build()
        _split_multi_waits(nc)
        _NC_CACHE = nc
    return _NC_CACHE


def _prep_core_inputs(core, xt, xs, Wt, bt, Ws, bs, Wc, bc, lam_ts, lam_st,
                      lam_ss):
    b, hg = core // HPC, core % HPC
    c0 = hg * HPC * D  # 256*hg
    lts, lst, lss = float(lam_ts[0]), float(lam_st[0]), float(lam_ss[0])

    wq_full = Wt[:, c0:c0 + HPC * D]                     # (E, 256) qt
    wqs_full = Ws[:, c0:c0 + HPC * D]                    # (E, 256) qs
    wv_full = Wt[:, 2 * E + c0:2 * E + c0 + HPC * D]     # (E, 256)
    ktw = Wt[:, E + c0:E + c0 + HPC * D]                 # (E, 256)
    ksw = Ws[:, E + c0:E + c0 + HPC * D]                 # (E, 256)


    def chunked(a, nk, dtype=np.float32):
        return np.ascontiguousarray(
            a.reshape(nk, 128, a.shape[1]).transpose(1, 0, 2)).astype(dtype)

    btq = bt[c0:c0 + HPC * D]
    bsq = bs[c0:c0 + HPC * D]
    btk = bt[E + c0:E + c0 + HPC * D]
    bsk = bs[E + c0:E + c0 + HPC * D]
    bq_arr = np.zeros((128, 2), np.float32)
    bqs_arr = np.zeros((128, 2), np.float32)
    bk1_arr = np.zeros((128, 2), np.float32)
    bk2_arr = np.zeros((128, 2), np.float32)
    for j in range(2):
        bq_arr[0:64, j] = btq[(2 * j) * D:(2 * j + 1) * D]
        bq_arr[64:128, j] = btq[(2 * j + 1) * D:(2 * j + 2) * D]
        bqs_arr[0:64, j] = bsq[(2 * j) * D:(2 * j + 1) * D]
        bqs_arr[64:128, j] = bsq[(2 * j + 1) * D:(2 * j + 2) * D]
    for j in range(2):
        h0, h1 = 2 * j, 2 * j + 1
        bk1_arr[0:64, j] = btk[h0 * D:(h0 + 1) * D] + lts * bsk[h0 * D:(h0 + 1) * D] * 0
        bk1_arr[64:128, j] = btk[h1 * D:(h1 + 1) * D] + lts * bsk[h1 * D:(h1 + 1) * D] * 0
        bk2_arr[0:64, j] = lst * btk[h0 * D:(h0 + 1) * D]
        bk2_arr[64:128, j] = lst * btk[h1 * D:(h1 + 1) * D]
    # note: bsk folded via ks having no bias -> fold lam*bsk into bk arrays
    for j in range(2):
        h0, h1 = 2 * j, 2 * j + 1
        bk1_arr[0:64, j] = btk[h0 * D:(h0 + 1) * D] + lts * bsk[h0 * D:(h0 + 1) * D]
        bk1_arr[64:128, j] = btk[h1 * D:(h1 + 1) * D] + lts * bsk[h1 * D:(h1 + 1) * D]
        bk2_arr[0:64, j] = lst * btk[h0 * D:(h0 + 1) * D] + lss * bsk[h0 * D:(h0 + 1) * D]
        bk2_arr[64:128, j] = lst * btk[h1 * D:(h1 + 1) * D] + lss * bsk[h1 * D:(h1 + 1) * D]

    return {
        "xtT": np.ascontiguousarray(xt[b].T).astype(np.float16),
        "xsT": np.ascontiguousarray(xs[b].T).astype(np.float16),
        "wq": chunked(wq_full, EC, np.float16),
        "wqs": chunked(wqs_full, EC, np.float16),
        "wkt": chunked(ktw, EC, np.float16),
        "wks": chunked(ksw, EC, np.float16),
        "wv": chunked(wv_full, EC, np.float16),
        "wc": chunked(Wc[c0:c0 + HPC * D, :], 2, np.float16),
        "bq": bq_arr,
        "bqs": bqs_arr,
        "bk1": bk1_arr,
        "bk2": bk2_arr,
        "lamv": np.tile(np.array([[lts, lst, lss]], np.float32), (128, 1)),
        "ones": np.ones((128, NCHUNK, 2, 64), np.float16),
    }


def kernel(**inputs):
    xt = np.asarray(inputs["xt"], np.float32)
    xs = np.asarray(inputs["xs"], np.float32)
    Wc = np.asarray(inputs["Wc"], np.float32)
    bt = np.asarray(inputs["bt"], np.float32)
    bc = np.asarray(inputs["bc"], np.float32)
    args = dict(
        xt=xt, xs=xs,
        Wt=np.asarray(inputs["Wt"], np.float32),
        bt=bt,
        Ws=np.asarray(inputs["Ws"], np.float32),
        bs=np.asarray(inputs["bs"], np.float32),
        Wc=Wc, bc=bc,
        lam_ts=np.asarray(inputs["lam_ts"], np.float32),
        lam_st=np.asarray(inputs["lam_st"], np.float32),
        lam_ss=np.asarray(inputs["lam_ss"], np.float32),
    )
    in_maps = [_prep_core_inputs(c, **args) for c in range(NCORES)]
    nc = _get_nc()
    res = run_bass_kernel_spmd(nc, in_maps, list(range(NCORES)))
    out = np.zeros((B, L, E), np.float32)
    for c in range(NCORES):
        out[c // HPC] += res.results[c]["out"]
    # v-bias and c-bias folded in on the host: softmax rows sum to one, so
    # the v bias contributes bv @ Wc (a constant row) to every position.
    out += bt[2 * E:] @ Wc + bc
    return out


# revision 29
# speedup vs baseline: 1.0256x; 1.0256x over previous
"""Disentangled spatial attention TRN2 kernel (8 NeuronCores).

Sharding: 8 cores = 2 batches x 4 head-groups (4 heads each).
Per core, transposed-activation layout:
  qcat[h] (128, L):  rows 0:64 qt_h, rows 64:128 qs_h
  kcat[h] (128, L):  rows 0:64 k1_h = kt + lam_ts*ks,
                     rows 64:128 k2_h = lam_st*kt + lam_ss*ks
  scores^T chunk = kcat_chunk.T @ qcat  (both reference score einsums
  fused into one K=128 matmul; lam_* folded into weight shards on host)
  softmax row-sums ride along the PV matmul as 64 replicated "ones"
  columns of the v operand; normalization happens on the way into the
  transposed y layout that feeds the output projection.
All matmul operands are fp16 (same PE rate as bf16, 8x lower rounding
error; accumulation is fp32 in PSUM).  kt/ks are computed once per head
pair and combined with the lam scalars on DVE/ACT.  v/c biases are
folded in on the host (exact: softmax rows sum to 1), qkv biases are
added on device.  Partition-base moves use SBUF->SBUF DMA (compute
engines are lane-locked); kcat/qcat staging DMAs ride the idle gpsimd
SWDGE queue to keep the Sync sequencer free.
"""
import numpy as np
import ml_dtypes
import concourse.bass as bass
import concourse.mybir as mybir
import concourse.tile as tile
from concourse.bass_utils import run_bass_kernel_spmd

F32 = mybir.dt.float32
F32R = mybir.dt.float32r
BF16 = mybir.dt.float16  # fp16: same PE rate as bf16, 8x lower rounding error
AF = mybir.ActivationFunctionType

B, L, E, H, D = 2, 2048, 1024, 16, 64
HPC = 4          # heads per core
NCORES = 8
LTB = 512        # L block for phase 1
NLTB = L // LTB  # 4
NCHUNK = L // 128  # 16 Lk chunks
EC = E // 128    # 8 E chunks


def _split_multi_waits(nc, max_waits=1):
    """walrus codegen allows only one sync wait per instruction; move extra
    waits onto standalone same-engine NoOps placed just before."""
    n_split = 0
    for f in nc.m.functions:
        for blk in f.blocks:
            insts = list(blk.instructions)
            out = []
            changed = False
            for inst in insts:
                si = inst.sync_info
                waits = list(si.on_wait) if si is not None and si.on_wait else []
                if len(waits) > max_waits:
                    keep = waits[-max_waits:]
                    extra = waits[:-max_waits]
                    for w in extra:
                        nop = mybir.InstNoOp(
                            name=f"{inst.name}-wsplit{n_split}",
                            engine=inst.engine,
                            ins=[], outs=[],
                            sync_info=mybir.SyncInfo(on_wait=[w], on_update=[]),
                        )
                        out.append(nop)
                        n_split += 1
                    inst.sync_info = mybir.SyncInfo(
                        on_wait=keep,
                        on_update=list(si.on_update) if si.on_update else [],
                    )
                    changed = True
                out.append(inst)
            if changed:
                blk.instructions = out
    return n_split


def _build():
    nc = bass.Bass()
    xtT = nc.declare_dram_parameter("xtT", [E, L], BF16, isOutput=False)
    xsT = nc.declare_dram_parameter("xsT", [E, L], BF16, isOutput=False)
    wq = nc.declare_dram_parameter("wq", [128, EC, HPC * D], BF16, isOutput=False)
    wqs = nc.declare_dram_parameter("wqs", [128, EC, HPC * D], BF16, isOutput=False)
    wkt = nc.declare_dram_parameter("wkt", [128, EC, HPC * D], BF16, isOutput=False)
    wks = nc.declare_dram_parameter("wks", [128, EC, HPC * D], BF16, isOutput=False)
    wv = nc.declare_dram_parameter("wv", [128, EC, HPC * D], BF16, isOutput=False)
    wc = nc.declare_dram_parameter("wc", [128, 2, E], BF16, isOutput=False)
    bq = nc.declare_dram_parameter("bq", [128, 2], F32, isOutput=False)
    bqs = nc.declare_dram_parameter("bqs", [128, 2], F32, isOutput=False)
    bk1 = nc.declare_dram_parameter("bk1", [128, 2], F32, isOutput=False)
    bk2 = nc.declare_dram_parameter("bk2", [128, 2], F32, isOutput=False)
    lamv = nc.declare_dram_parameter("lamv", [128, 3], F32, isOutput=False)
    ones = nc.declare_dram_parameter("ones", [128, NCHUNK, 2, 64], BF16,
                                     isOutput=False)
    out = nc.declare_dram_parameter("out", [L, E], F32, isOutput=True)

    xtT_v = xtT.rearrange("(k p) l -> p k l", p=128)   # (128, 8, L)
    xsT_v = xsT.rearrange("(k p) l -> p k l", p=128)

    with tile.TileContext(nc) as tc:
        with tc.tile_pool(name="wpool", bufs=1) as wpool, \
             tc.tile_pool(name="persist", bufs=1) as pp:
            qcat = [pp.tile([128, L], BF16, tag=f"qcat{h}", name=f"qcat{h}")
                    for h in range(HPC)]
            kcat = [pp.tile([128, L], BF16, tag=f"kcat{h}", name=f"kcat{h}")
                    for h in range(HPC)]
            # v_aug: (128, chunk, head, 128); head slot s=0: [ones | v],
            # s=1: [v | ones]
            v_sb = pp.tile([128, NCHUNK, HPC, 128], BF16, name="v_sb")
            yT = [pp.tile([128, L], BF16, tag=f"yT{j}", name=f"yT{j}")
                  for j in range(2)]
            xt_sb = pp.tile([128, EC, L], BF16, name="xt_sb")
            xs_sb = pp.tile([128, EC, L], BF16, name="xs_sb")

            wq_sb = wpool.tile([128, EC, HPC * D], BF16)
            wqs_sb = wpool.tile([128, EC, HPC * D], BF16)
            wkt_sb = wpool.tile([128, EC, HPC * D], BF16)
            wks_sb = wpool.tile([128, EC, HPC * D], BF16)
            wv_sb = wpool.tile([128, EC, HPC * D], BF16)
            bq_sb = wpool.tile([128, 2], F32)
            bqs_sb = wpool.tile([128, 2], F32)
            bk1_sb = wpool.tile([128, 2], F32)
            bk2_sb = wpool.tile([128, 2], F32)
            lam_sb = wpool.tile([128, 3], F32)
            wc_sb = wpool.tile([128, 2, E], BF16)

            nc.sync.dma_start(wv_sb[:], wv[:])
            for xc in range(4):
                xls = slice(xc * 512, (xc + 1) * 512)
                nc.sync.dma_start(xt_sb[:, :, xls], xtT_v[:, :, xls])
            nc.sync.dma_start(wkt_sb[:], wkt[:])
            nc.sync.dma_start(wks_sb[:], wks[:])
            nc.sync.dma_start(lam_sb[:], lamv[:])
            nc.sync.dma_start(bk1_sb[:], bk1[:])
            nc.sync.dma_start(bk2_sb[:], bk2[:])
            for xc in range(4):
                xls = slice(xc * 512, (xc + 1) * 512)
                nc.sync.dma_start(xs_sb[:, :, xls], xsT_v[:, :, xls])
            nc.sync.dma_start(wq_sb[:], wq[:])
            nc.sync.dma_start(bq_sb[:], bq[:])
            nc.sync.dma_start(wqs_sb[:], wqs[:])
            nc.sync.dma_start(bqs_sb[:], bqs[:])
            nc.sync.dma_start(v_sb[:, :, 0::2, 0:64], ones[:])
            nc.sync.dma_start(v_sb[:, :, 1::2, 64:128], ones[:])
            nc.sync.dma_start(wc_sb[:], wc[:])

            # ---- head pairs: QKV then attention, interleaved ----
            with tc.tile_pool(name="expp", bufs=6) as expp, \
                 tc.tile_pool(name="np2", bufs=2) as np2, \
                 tc.tile_pool(name="kcp", bufs=3) as kcp, \
                 tc.tile_pool(name="p2s", bufs=2, space="PSUM") as p2s, \
                 tc.tile_pool(name="p2y", bufs=2, space="PSUM") as p2y:
                pvp_cm = tc.tile_pool(name="pvp", bufs=3, space="PSUM")
                pvp = pvp_cm.__enter__()
                M_ = mybir.AluOpType.mult
                A_ = mybir.AluOpType.add

                def emit_ktks(j):
                    # kt/ks for the pair; combine into kcat (k1 | k2) with
                    # lam scalars; per-lt staged DMAs for partition shifts
                    for lt in range(4):
                        ls = slice(lt * 512, (lt + 1) * 512)
                        ktp = pvp.tile([128, 512], F32, tag="p1",
                                       name=f"ktp{j}{lt}")
                        for k in range(EC):
                            nc.tensor.matmul(
                                ktp[:], wkt_sb[:, k, j * 128:(j + 1) * 128],
                                xt_sb[:, k, ls],
                                start=(k == 0), stop=(k == EC - 1),
                                skip_group_check=True)
                        ksp = pvp.tile([128, 512], F32, tag="p1",
                                       name=f"ksp{j}{lt}")
                        for k in range(EC):
                            nc.tensor.matmul(
                                ksp[:], wks_sb[:, k, j * 128:(j + 1) * 128],
                                xs_sb[:, k, ls],
                                start=(k == 0), stop=(k == EC - 1),
                                skip_group_check=True)
                        kt1 = kcp.tile([128, 512], F32, tag="kt1",
                                       name=f"kt1{j}{lt}")
                        nc.scalar.activation(kt1[:], ktp[:], AF.Identity,
                                             bias=bk1_sb[:, j:j + 1])
                        kt2 = kcp.tile([128, 512], F32, tag="kt2",
                                       name=f"kt2{j}{lt}")
                        nc.scalar.activation(
                            kt2[:], ktp[:], AF.Identity,
                            bias=bk2_sb[:, j:j + 1], scale=lam_sb[:, 1:2])
                        # k1 (both heads) and k2 (both heads), full width
                        k1s = kcp.tile([128, 512], BF16, tag="k1s",
                                       name=f"k1s{j}{lt}")
                        nc.vector.scalar_tensor_tensor(
                            k1s[:], ksp[:], lam_sb[:, 0:1], kt1[:], M_, A_)
                        k2s = kcp.tile([128, 512], BF16, tag="k2s",
                                       name=f"k2s{j}{lt}")
                        nc.vector.scalar_tensor_tensor(
                            k2s[:], ksp[:], lam_sb[:, 2:3], kt2[:], M_, A_)
                        nc.gpsimd.dma_start(kcat[2 * j][0:64, ls], k1s[0:64, :])
                        nc.gpsimd.dma_start(kcat[2 * j + 1][0:64, ls],
                                            k1s[64:128, :])
                        nc.gpsimd.dma_start(kcat[2 * j][64:128, ls], k2s[0:64, :])
                        nc.gpsimd.dma_start(kcat[2 * j + 1][64:128, ls],
                                            k2s[64:128, :])

                def emit_q(j):
                    for lt in range(4):
                        ls = slice(lt * 512, (lt + 1) * 512)
                        pq = pvp.tile([128, 512], F32, tag="p1",
                                      name=f"pq{j}{lt}")
                        for k in range(EC):
                            nc.tensor.matmul(
                                pq[:], wq_sb[:, k, j * 128:(j + 1) * 128],
                                xt_sb[:, k, ls],
                                start=(k == 0), stop=(k == EC - 1),
                                skip_group_check=True)
                        nc.vector.tensor_scalar_add(
                            qcat[2 * j][0:64, ls], pq[0:64, :],
                            bq_sb[0:64, j:j + 1])
                        qst = kcp.tile([128, 512], BF16, tag="qst",
                                       name=f"qst{j}{lt}")
                        nc.vector.tensor_scalar_add(
                            qst[64:128, :], pq[64:128, :],
                            bq_sb[64:128, j:j + 1])
                        nc.gpsimd.dma_start(qcat[2 * j + 1][0:64, ls],
                                            qst[64:128, :])
                    for lt in range(4):
                        ls = slice(lt * 512, (lt + 1) * 512)
                        pq = pvp.tile([128, 512], F32, tag="p1",
                                      name=f"pqs{j}{lt}")
                        for k in range(EC):
                            nc.tensor.matmul(
                                pq[:], wqs_sb[:, k, j * 128:(j + 1) * 128],
                                xs_sb[:, k, ls],
                                start=(k == 0), stop=(k == EC - 1),
                                skip_group_check=True)
                        qst = kcp.tile([128, 512], BF16, tag="qst",
                                       name=f"qsst{j}{lt}")
                        nc.scalar.activation(
                            qst[0:64, :], pq[0:64, :], AF.Identity,
                            bias=bqs_sb[0:64, j:j + 1])
                        nc.gpsimd.dma_start(qcat[2 * j][64:128, ls],
                                            qst[0:64, :])
                        nc.scalar.activation(
                            qcat[2 * j + 1][64:128, ls], pq[64:128, :],
                            AF.Identity, bias=bqs_sb[64:128, j:j + 1])

                def emit_v():
                    for ck in range(NCHUNK):
                        pv = pvp.tile([128, HPC * D], F32, tag="p1",
                                      name=f"pv{ck}")
                        for k in range(EC):
                            nc.tensor.matmul(
                                pv[:], xt_sb[:, k, ck * 128:(ck + 1) * 128],
                                wv_sb[:, k, :],
                                start=(k == 0), stop=(k == EC - 1),
                                skip_group_check=True)
                        pv_v = pv.rearrange("p (h d) -> p h d", d=D)
                        nc.vector.tensor_copy(v_sb[:, ck, 0::2, 64:128],
                                              pv_v[:, 0::2, :])
                        nc.vector.tensor_copy(v_sb[:, ck, 1::2, 0:64],
                                              pv_v[:, 1::2, :])

                def emit_attn(h, lqs_list=range(4)):
                    j, s = h // 2, h % 2
                    sums_h = slice(0, 64) if s == 0 else slice(64, 128)
                    y_h = slice(64, 128) if s == 0 else slice(0, 64)
                    slot = slice(0, 64) if s == 0 else slice(64, 128)
                    for lq in lqs_list:
                        qs_ = slice(lq * 512, (lq + 1) * 512)
                        py = p2y.tile([128, 512], F32, tag="py", bufs=1,
                                      name=f"py{h}{lq}")
                        for g in range(8):
                            ps = p2s.tile([128, 1024], F32, tag="ps",
                                          name=f"ps{h}{lq}{g}")
                            for hf in range(2):
                                ck = 2 * g + hf
                                nc.tensor.matmul(
                                    ps[:, hf * 512:(hf + 1) * 512],
                                    kcat[h][:, ck * 128:(ck + 1) * 128],
                                    qcat[h][:, qs_],
                                    start=True, stop=True,
                                    skip_group_check=True)
                            ex = expp.tile([128, 1024], BF16, tag="ex",
                                           name=f"ex{h}{lq}{g}")
                            nc.scalar.activation(ex[:], ps[:], AF.Exp,
                                                 scale=0.125)
                            for hf in range(2):
                                ck = 2 * g + hf
                                nc.tensor.matmul(
                                    py[:], v_sb[:, ck, h, :],
                                    ex[:, hf * 512:(hf + 1) * 512],
                                    start=(ck == 0), stop=(ck == NCHUNK - 1),
                                    skip_group_check=True)
                        ysb = np2.tile([128, 512], F32, tag="ysb",
                                       name=f"ysb{h}{lq}")
                        rec = np2.tile([128, 512], F32, tag="rec",
                                       name=f"rec{h}{lq}")
                        nc.vector.tensor_copy(ysb[:], py[:])
                        if h == 3:
                            lnt = np2.tile([128, 512], F32, tag="lnt",
                                           name=f"ln{h}{lq}")
                            nc.scalar.activation(lnt[sums_h, :],
                                                 ysb[sums_h, :], AF.Ln)
                            nc.scalar.activation(rec[sums_h, :],
                                                 lnt[sums_h, :], AF.Exp,
                                                 scale=-1.0)
                        else:
                            nc.vector.reciprocal(rec[sums_h, :],
                                                 ysb[sums_h, :])
                        rec2 = np2.tile([128, 512], F32, tag="rec2",
                                        name=f"rec2{h}{lq}")
                        nc.sync.dma_start(rec2[y_h, :], rec[sums_h, :])
                        yst = np2.tile([128, 512], BF16, tag="yst",
                                       name=f"yst{h}{lq}")
                        nc.vector.tensor_tensor(yst[y_h, :], ysb[y_h, :],
                                                rec2[y_h, :],
                                                mybir.AluOpType.mult)
                        nc.sync.dma_start(yT[j][slot, qs_], yst[y_h, :])

                emit_v()
                emit_ktks(0)
                emit_q(0)
                emit_attn(0)
                emit_attn(1)
                emit_ktks(1)
                emit_q(1)
                pvp_cm.__exit__(None, None, None)
                emit_attn(2)

                with tc.tile_pool(name="outp", bufs=3) as outp, \
                     tc.tile_pool(name="p3o", bufs=2, space="PSUM") as p3o:
                    def emit_proj(lq):
                        for lqt in range(lq * 4, (lq + 1) * 4):
                            lqs = slice(lqt * 128, (lqt + 1) * 128)
                            ot = outp.tile([128, E], F32, tag="ot",
                                           name=f"ot{lqt}")
                            for nch in range(2):
                                ns = slice(nch * 512, (nch + 1) * 512)
                                po = p3o.tile([128, 512], F32, tag="po",
                                              name=f"po{lqt}{nch}")
                                nc.tensor.matmul(po[:], yT[0][:, lqs],
                                                 wc_sb[:, 0, ns],
                                                 start=True, stop=False,
                                                 skip_group_check=True)
                                nc.tensor.matmul(po[:], yT[1][:, lqs],
                                                 wc_sb[:, 1, ns],
                                                 start=False, stop=True,
                                                 skip_group_check=True)
                                if nch == 0:
                                    nc.scalar.copy(ot[:, ns], po[:])
                                else:
                                    nc.vector.tensor_copy(ot[:, ns], po[:])
                            nc.sync.dma_start(out[lqs, :], ot[:])

                    for lq in range(4):
                        emit_attn(3, [lq])
                        emit_proj(lq)

    return nc


_NC_CACHE = None


def _get_nc():
    global _NC_CACHE
    if _NC_CACHE is None:
        nc = _build()
        _split_multi_waits(nc)
        _NC_CACHE = nc
    return _NC_CACHE


def _prep_core_inputs(core, xt, xs, Wt, bt, Ws, bs, Wc, bc, lam_ts, lam_st,
                      lam_ss):
    b, hg = core // HPC, core % HPC
    c0 = hg * HPC * D  # 256*hg
    lts, lst, lss = float(lam_ts[0]), float(lam_st[0]), float(lam_ss[0])

    wq_full = Wt[:, c0:c0 + HPC * D]                     # (E, 256) qt
    wqs_full = Ws[:, c0:c0 + HPC * D]                    # (E, 256) qs
    wv_full = Wt[:, 2 * E + c0:2 * E + c0 + HPC * D]     # (E, 256)
    ktw = Wt[:, E + c0:E + c0 + HPC * D]                 # (E, 256)
    ksw = Ws[:, E + c0:E + c0 + HPC * D]                 # (E, 256)


    def chunked(a, nk, dtype=np.float32):
        return np.ascontiguousarray(
            a.reshape(nk, 128, a.shape[1]).transpose(1, 0, 2)).astype(dtype)

    btq = bt[c0:c0 + HPC * D]
    bsq = bs[c0:c0 + HPC * D]
    btk = bt[E + c0:E + c0 + HPC * D]
    bsk = bs[E + c0:E + c0 + HPC * D]
    bq_arr = np.zeros((128, 2), np.float32)
    bqs_arr = np.zeros((128, 2), np.float32)
    bk1_arr = np.zeros((128, 2), np.float32)
    bk2_arr = np.zeros((128, 2), np.float32)
    for j in range(2):
        bq_arr[0:64, j] = btq[(2 * j) * D:(2 * j + 1) * D]
        bq_arr[64:128, j] = btq[(2 * j + 1) * D:(2 * j + 2) * D]
        bqs_arr[0:64, j] = bsq[(2 * j) * D:(2 * j + 1) * D]
        bqs_arr[64:128, j] = bsq[(2 * j + 1) * D:(2 * j + 2) * D]
    for j in range(2):
        h0, h1 = 2 * j, 2 * j + 1
        bk1_arr[0:64, j] = btk[h0 * D:(h0 + 1) * D] + lts * bsk[h0 * D:(h0 + 1) * D] * 0
        bk1_arr[64:128, j] = btk[h1 * D:(h1 + 1) * D] + lts * bsk[h1 * D:(h1 + 1) * D] * 0
        bk2_arr[0:64, j] = lst * btk[h0 * D:(h0 + 1) * D]
        bk2_arr[64:128, j] = lst * btk[h1 * D:(h1 + 1) * D]
    # note: bsk folded via ks having no bias -> fold lam*bsk into bk arrays
    for j in range(2):
        h0, h1 = 2 * j, 2 * j + 1
        bk1_arr[0:64, j] = btk[h0 * D:(h0 + 1) * D] + lts * bsk[h0 * D:(h0 + 1) * D]
        bk1_arr[64:128, j] = btk[h1 * D:(h1 + 1) * D] + lts * bsk[h1 * D:(h1 + 1) * D]
        bk2_arr[0:64, j] = lst * btk[h0 * D:(h0 + 1) * D] + lss * bsk[h0 * D:(h0 + 1) * D]
        bk2_arr[64:128, j] = lst * btk[h1 * D:(h1 + 1) * D] + lss * bsk[h1 * D:(h1 + 1) * D]

    return {
        "xtT": np.ascontiguousarray(xt[b].T).astype(np.float16),
        "xsT": np.ascontiguousarray(xs[b].T).astype(np.float16),
        "wq": chunked(wq_full, EC, np.float16),
        "wqs": chunked(wqs_full, EC, np.float16),
        "wkt": chunked(ktw, EC, np.float16),
        "wks": chunked(ksw, EC, np.float16),
        "wv": chunked(wv_full, EC, np.float16),
        "wc": chunked(Wc[c0:c0 + HPC * D, :], 2, np.float16),
        "bq": bq_arr,
        "bqs": bqs_arr,
        "bk1": bk1_arr,
        "bk2": bk2_arr,
        "lamv": np.tile(np.array([[lts, lst, lss]], np.float32), (128, 1)),
        "ones": np.ones((128, NCHUNK, 2, 64), np.float16),
    }


def kernel(**inputs):
    xt = np.asarray(inputs["xt"], np.float32)
    xs = np.asarray(inputs["xs"], np.float32)
    Wc = np.asarray(inputs["Wc"], np.float32)
    bt = np.asarray(inputs["bt"], np.float32)
    bc = np.asarray(inputs["bc"], np.float32)
    args = dict(
        xt=xt, xs=xs,
        Wt=np.asarray(inputs["Wt"], np.float32),
        bt=bt,
        Ws=np.asarray(inputs["Ws"], np.float32),
        bs=np.asarray(inputs["bs"], np.float32),
        Wc=Wc, bc=bc,
        lam_ts=np.asarray(inputs["lam_ts"], np.float32),
        lam_st=np.asarray(inputs["lam_st"], np.float32),
        lam_ss=np.asarray(inputs["lam_ss"], np.float32),
    )
    in_maps = [_prep_core_inputs(c, **args) for c in range(NCORES)]
    nc = _get_nc()
    res = run_bass_kernel_spmd(nc, in_maps, list(range(NCORES)))
    out = np.zeros((B, L, E), np.float32)
    for c in range(NCORES):
        out[c // HPC] += res.results[c]["out"]
    # v-bias and c-bias folded in on the host: softmax rows sum to one, so
    # the v bias contributes bv @ Wc (a constant row) to every position.
    out += bt[2 * E:] @ Wc + bc
    return out
